# revision 2
# baseline (speedup 1.0000x reference)
"""MoE routing kernel for Trainium2 (8 NeuronCores, expert-parallel).

Problem: y[n] = x[n] @ W[index[n]].T + b[index[n]]
  x [16384, 1024] f32, index [16384] i32, W [8, 512, 1024] f32, b [8, 512] f32

Strategy (expert-parallel, dispatch on index during sharding):
  Core e owns expert e. The host groups rows by expert (the all-to-all
  dispatch), packs each core's rows into PE-friendly transposed tiles, and
  each core runs a dense [R,1024] @ [1024,512] matmul with its expert's
  weights. Results are scattered back to original row order on the host.

Device layout per core (one NEFF, SPMD on cores 0-7):
  xT  [RT, 8, 128, 128]  (row-tile, k-tile, k, r)  — lhsT blocks (stationary)
  wT  [8, 128, 512]      (k-tile, k, o)            — rhs blocks (moving)
  y   [RT, 128, 512]     (row-tile, r, o)
  For each row-tile: accumulate 8 matmuls over k-tiles into one PSUM bank,
  copy PSUM->SBUF on DVE, DMA out.
"""

from contextlib import ExitStack

import numpy as np

import concourse.bass as bass
import concourse.mybir as mybir
import concourse.tile as tile
from concourse import bacc
from concourse.bass_utils import run_bass_kernel_spmd

N_CORES = 8
D_IN = 1024
D_OUT = 512
KT = D_IN // 128  # 8 k-tiles

# matmul input dtype: float32r streams 1 column/cycle (vs 4 cycles for
# float32) at N>=256; bits are identical to fp32.
MM_DT = mybir.dt.float32r


def build_nc(rt: int, mm_dt=MM_DT):
    """Build + compile the per-core Bass program for `rt` row-tiles."""
    nc = bacc.Bacc(
        "TRN2", target_bir_lowering=False, debug=False, num_devices=N_CORES
    )
    f32 = mybir.dt.float32
    xT = nc.dram_tensor("xT", [rt, KT, 128, 128], mm_dt, kind="ExternalInput").ap()
    wT = nc.dram_tensor("wT", [KT, 128, D_OUT], mm_dt, kind="ExternalInput").ap()
    y = nc.dram_tensor("y", [rt, 128, D_OUT], f32, kind="ExternalOutput").ap()

    with tile.TileContext(nc) as tc, ExitStack() as ctx:
        w_pool = ctx.enter_context(tc.tile_pool(name="w", bufs=1))
        x_pool = ctx.enter_context(tc.tile_pool(name="x", bufs=4))
        o_pool = ctx.enter_context(tc.tile_pool(name="o", bufs=4))
        p_pool = ctx.enter_context(tc.tile_pool(name="p", bufs=4, space="PSUM"))

        # Per-k-tile weight tiles so matmuls can start as soon as their
        # own k-slice of W has landed.
        w_tiles = []
        for kt in range(KT):
            w_t = w_pool.tile([128, D_OUT], mm_dt, tag=f"w{kt}")
            nc.sync.dma_start(w_t[:], wT[kt])
            w_tiles.append(w_t)

        for r in range(rt):
            x_t = x_pool.tile([128, KT * 128], mm_dt, tag="x")
            nc.sync.dma_start(
                x_t[:].rearrange("k (kt r) -> k kt r", kt=KT),
                xT[r].rearrange("kt k r -> k kt r"),
            )
            psum = p_pool.tile([128, D_OUT], f32, tag="ps")
            for kt in range(KT):
                nc.tensor.matmul(
                    psum[:],
                    x_t[:, bass.ts(kt, 128)],
                    w_tiles[kt][:],
                    start=(kt == 0),
                    stop=(kt == KT - 1),
                )
            o_t = o_pool.tile([128, D_OUT], f32, tag="o")
            nc.vector.tensor_copy(o_t[:], psum[:])
            nc.sync.dma_start(y[r], o_t[:])

    nc.compile()
    return nc


def make_in_maps(x, index, W):
    """Group rows by expert, pack per-core transposed tiles.

    Returns (in_maps, rows_per_expert, rt) where rows_per_expert[e] is the
    original row indices handled by core e.
    """
    x = np.ascontiguousarray(x, dtype=np.float32)
    W = np.ascontiguousarray(W, dtype=np.float32)
    rows_per_expert = [np.nonzero(index == e)[0] for e in range(N_CORES)]
    max_rows = max(len(r) for r in rows_per_expert)
    rt = max((max_rows + 127) // 128, 1)
    r_pad = rt * 128

    in_maps = []
    for e in range(N_CORES):
        rows = rows_per_expert[e]
        xp = np.zeros((r_pad, D_IN), np.float32)
        xp[: len(rows)] = x[rows]
        # [R, D_IN] -> [RT, 128r, KT, 128k] -> [RT, KT, 128k, 128r]
        xT = np.ascontiguousarray(
            xp.reshape(rt, 128, KT, 128).transpose(0, 2, 3, 1)
        )
        wT = np.ascontiguousarray(W[e].T.reshape(KT, 128, D_OUT))
        in_maps.append({"xT": xT, "wT": wT})
    return in_maps, rows_per_expert, rt


def assemble_output(results, rows_per_expert, n_rows, index=None, b=None):
    y = np.zeros((n_rows, D_OUT), np.float32)
    for e, rows in enumerate(rows_per_expert):
        yc = results[e]["y"].reshape(-1, D_OUT)
        y[rows] = yc[: len(rows)]
    if b is not None and np.any(b):
        y += np.asarray(b, np.float32)[np.asarray(index)]
    return y


def kernel(x, index, W, b):
    x = np.asarray(x)
    index = np.asarray(index, np.int32)
    W = np.asarray(W)
    b = np.asarray(b)
    in_maps, rows_per_expert, rt = make_in_maps(x, index, W)
    nc = build_nc(rt)
    res = run_bass_kernel_spmd(nc, in_maps, core_ids=list(range(N_CORES)))
    return assemble_output(res.results, rows_per_expert, x.shape[0], index, b)


# revision 7
# speedup vs baseline: 1.0963x; 1.0963x over previous
"""MoE routing kernel for Trainium2 (8 NeuronCores, expert-parallel).

Problem: y[n] = x[n] @ W[index[n]].T + b[index[n]]
  x [16384, 1024] f32, index [16384] i32, W [8, 512, 1024] f32, b [8, 512] f32

Strategy (expert-parallel, dispatch on index during sharding):
  Core e owns expert e. The host groups rows by expert (the all-to-all
  dispatch), packs each core's rows into PE-friendly transposed tiles, and
  each core runs a dense [R,1024] @ [1024,512] matmul with its expert's
  weights. Results are scattered back to original row order on the host.

Device layout per core (one NEFF, SPMD on cores 0-7):
  xT  [RT, 128, 8, 128]  (row-tile, k%128, k-tile, r) — lhsT blocks; the
                         partition line (fixed k) is 4KB contiguous in DRAM
  wT  [8, 128, 512]      (k-tile, k, o)               — rhs blocks (moving)
  y   [RT, 128, 512]     (row-tile, r, o)
  For each row-tile: accumulate 8 matmuls over k-tiles into one PSUM bank,
  copy PSUM->SBUF on DVE, DMA out.
"""

from contextlib import ExitStack

import numpy as np

import concourse.bass as bass
import concourse.mybir as mybir
import concourse.tile as tile
from concourse import bacc
from concourse.bass_utils import run_bass_kernel_spmd

N_CORES = 8
D_IN = 1024
D_OUT = 512
KT = D_IN // 128  # 8 k-tiles

# matmul input dtype: float32r streams 1 column/cycle (vs 4 cycles for
# float32) at N>=256; bits are identical to fp32.
MM_DT = mybir.dt.float32r


def build_nc(rt: int, mm_dt=MM_DT):
    """Build + compile the per-core Bass program for `rt` row-tiles."""
    nc = bacc.Bacc(
        "TRN2", target_bir_lowering=False, debug=False, num_devices=N_CORES
    )
    f32 = mybir.dt.float32
    xT = nc.dram_tensor("xT", [rt, 128, KT * 128], mm_dt, kind="ExternalInput").ap()
    wT = nc.dram_tensor("wT", [KT, 128, D_OUT], mm_dt, kind="ExternalInput").ap()
    y = nc.dram_tensor("y", [rt, 128, D_OUT], f32, kind="ExternalOutput").ap()

    with tile.TileContext(nc) as tc, ExitStack() as ctx:
        w_pool = ctx.enter_context(tc.tile_pool(name="w", bufs=1))
        x_pool = ctx.enter_context(tc.tile_pool(name="x", bufs=6))
        o_pool = ctx.enter_context(tc.tile_pool(name="o", bufs=6))
        p_pool = ctx.enter_context(tc.tile_pool(name="p", bufs=6, space="PSUM"))

        # Per-k-tile weight tiles so matmuls can start as soon as their
        # own k-slice of W has landed.
        w_tiles = []
        for kt in range(KT):
            w_t = w_pool.tile([128, D_OUT], mm_dt, tag=f"w{kt}")
            nc.sync.dma_start(w_t[:], wT[kt])
            w_tiles.append(w_t)

        for r in range(rt):
            x_t = x_pool.tile([128, KT * 128], mm_dt, tag="x")
            nc.sync.dma_start(x_t[:], xT[r])
            psum = p_pool.tile([128, D_OUT], f32, tag="ps")
            for kt in range(KT):
                nc.tensor.matmul(
                    psum[:],
                    x_t[:, bass.ts(kt, 128)],
                    w_tiles[kt][:],
                    start=(kt == 0),
                    stop=(kt == KT - 1),
                )
            o_t = o_pool.tile([128, D_OUT], f32, tag="o")
            nc.vector.tensor_copy(o_t[:], psum[:])
            nc.sync.dma_start(y[r], o_t[:])

    nc.compile()
    return nc


def make_in_maps(x, index, W):
    """Group rows by expert, pack per-core transposed tiles.

    Returns (in_maps, rows_per_expert, rt) where rows_per_expert[e] is the
    original row indices handled by core e.
    """
    x = np.ascontiguousarray(x, dtype=np.float32)
    W = np.ascontiguousarray(W, dtype=np.float32)
    rows_per_expert = [np.nonzero(index == e)[0] for e in range(N_CORES)]
    max_rows = max(len(r) for r in rows_per_expert)
    rt = max((max_rows + 127) // 128, 1)
    r_pad = rt * 128

    in_maps = []
    for e in range(N_CORES):
        rows = rows_per_expert[e]
        xp = np.zeros((r_pad, D_IN), np.float32)
        xp[: len(rows)] = x[rows]
        # [R, D_IN] -> [RT, 128r, KT, 128k] -> [RT, 128k, KT, 128r]
        # so a partition line (fixed k) is KT*128*4B = 4KB contiguous.
        xT = np.ascontiguousarray(
            xp.reshape(rt, 128, KT, 128).transpose(0, 3, 2, 1).reshape(rt, 128, -1)
        )
        wT = np.ascontiguousarray(W[e].T.reshape(KT, 128, D_OUT))
        in_maps.append({"xT": xT, "wT": wT})
    return in_maps, rows_per_expert, rt


def assemble_output(results, rows_per_expert, n_rows, index=None, b=None):
    y = np.zeros((n_rows, D_OUT), np.float32)
    for e, rows in enumerate(rows_per_expert):
        yc = results[e]["y"].reshape(-1, D_OUT)
        y[rows] = yc[: len(rows)]
    if b is not None and np.any(b):
        y += np.asarray(b, np.float32)[np.asarray(index)]
    return y


def kernel(x, index, W, b):
    x = np.asarray(x)
    index = np.asarray(index, np.int32)
    W = np.asarray(W)
    b = np.asarray(b)
    in_maps, rows_per_expert, rt = make_in_maps(x, index, W)
    nc = build_nc(rt)
    res = run_bass_kernel_spmd(nc, in_maps, core_ids=list(range(N_CORES)))
    return assemble_output(res.results, rows_per_expert, x.shape[0], index, b)


# revision 11
# speedup vs baseline: 1.5193x; 1.3858x over previous
"""MoE routing kernel for Trainium2 (8 NeuronCores, expert-parallel).

Problem: y[n] = x[n] @ W[index[n]].T + b[index[n]]
  x [16384, 1024] f32, index [16384] i32, W [8, 512, 1024] f32, b [8, 512] f32

Strategy (expert-parallel, dispatch on index during sharding):
  Core e owns expert e. The host groups rows by expert (the all-to-all
  dispatch), packs each core's rows into PE-friendly transposed tiles, and
  each core runs a dense [R,1024] @ [1024,512] matmul with its expert's
  weights. Results are scattered back to original row order on the host.

Device layout per core (one NEFF, SPMD on cores 0-7):
  xT  [RT, 128, 8, 128]  (row-tile, k%128, k-tile, r) — lhsT blocks; the
                         partition line (fixed k) is 4KB contiguous in DRAM
  wT  [8, 128, 512]      (k-tile, k, o)               — rhs blocks (moving)
  y   [RT, 128, 512]     (row-tile, r, o)
  For each row-tile: accumulate 8 matmuls over k-tiles into one PSUM bank,
  copy PSUM->SBUF on DVE, DMA out.
"""

from contextlib import ExitStack

import numpy as np

import concourse.bass as bass
import concourse.mybir as mybir
import concourse.tile as tile
from concourse import bacc
from concourse.bass_utils import run_bass_kernel_spmd

N_CORES = 8
D_IN = 1024
D_OUT = 512
KT = D_IN // 128  # 8 k-tiles

# matmul input dtypes (lhsT = x blocks, rhs = W blocks):
#   float32r streams 1 column/cycle (vs 4 cycles for float32) at N>=256;
#   bits are identical to fp32. bfloat16 additionally gets fast weight load.
X_DT = mybir.dt.float32r
W_DT = mybir.dt.float32r


def build_nc(rt: int, x_dt=None, w_dt=None):
    """Build + compile the per-core Bass program for `rt` row-tiles."""
    x_dt = x_dt or X_DT
    w_dt = w_dt or W_DT
    nc = bacc.Bacc(
        "TRN2", target_bir_lowering=False, debug=False, num_devices=N_CORES
    )
    f32 = mybir.dt.float32
    xT = nc.dram_tensor("xT", [rt, 128, KT * 128], x_dt, kind="ExternalInput").ap()
    wT = nc.dram_tensor("wT", [KT, 128, D_OUT], w_dt, kind="ExternalInput").ap()
    y = nc.dram_tensor("y", [rt, 128, D_OUT], f32, kind="ExternalOutput").ap()

    with tile.TileContext(nc) as tc, ExitStack() as ctx:
        w_pool = ctx.enter_context(tc.tile_pool(name="w", bufs=1))
        x_pool = ctx.enter_context(tc.tile_pool(name="x", bufs=6))
        o_pool = ctx.enter_context(tc.tile_pool(name="o", bufs=6))
        p_pool = ctx.enter_context(tc.tile_pool(name="p", bufs=6, space="PSUM"))

        # W loads on the ACT HWDGE ring (nc.scalar) so they stream in
        # parallel with the x loads on the SP ring (nc.sync). Two halves so
        # the first matmuls can start after half of W has landed.
        w_tiles = []
        for kt in range(KT):
            w_tiles.append(
                w_pool.tile([128, D_OUT], w_dt, tag=f"w{kt}", name=f"w{kt}")
            )
        for h in (0, 1):
            half = KT // 2
            dst = [w_tiles[h * half + i] for i in range(half)]
            for i, w_t in enumerate(dst):
                nc.scalar.dma_start(w_t[:], wT[h * half + i])

        for r in range(rt):
            x_t = x_pool.tile([128, KT * 128], x_dt, tag="x")
            nc.sync.dma_start(x_t[:], xT[r])
            psum = p_pool.tile([128, D_OUT], f32, tag="ps")
            for kt in range(KT):
                nc.tensor.matmul(
                    psum[:],
                    x_t[:, bass.ts(kt, 128)],
                    w_tiles[kt][:],
                    start=(kt == 0),
                    stop=(kt == KT - 1),
                )
            o_t = o_pool.tile([128, D_OUT], f32, tag="o")
            nc.vector.tensor_copy(o_t[:], psum[:])
            nc.gpsimd.dma_start(y[r], o_t[:])

    nc.compile()
    return nc


def make_in_maps(x, index, W, x_dt=None, w_dt=None):
    """Group rows by expert, pack per-core transposed tiles.

    Returns (in_maps, rows_per_expert, rt) where rows_per_expert[e] is the
    original row indices handled by core e.
    """
    import concourse.mybir as _mybir

    x_np = _mybir.dt.np(x_dt or X_DT)
    w_np = _mybir.dt.np(w_dt or W_DT)
    x = np.ascontiguousarray(x, dtype=np.float32)
    W = np.ascontiguousarray(W, dtype=np.float32)
    rows_per_expert = [np.nonzero(index == e)[0] for e in range(N_CORES)]
    max_rows = max(len(r) for r in rows_per_expert)
    rt = max((max_rows + 127) // 128, 1)
    r_pad = rt * 128

    in_maps = []
    for e in range(N_CORES):
        rows = rows_per_expert[e]
        xp = np.zeros((r_pad, D_IN), np.float32)
        xp[: len(rows)] = x[rows]
        # [R, D_IN] -> [RT, 128r, KT, 128k] -> [RT, 128k, KT, 128r]
        # so a partition line (fixed k) is KT*128*4B = 4KB contiguous.
        xT = np.ascontiguousarray(
            xp.reshape(rt, 128, KT, 128).transpose(0, 3, 2, 1).reshape(rt, 128, -1),
            dtype=x_np,
        )
        wT = np.ascontiguousarray(W[e].T.reshape(KT, 128, D_OUT), dtype=w_np)
        in_maps.append({"xT": xT, "wT": wT})
    return in_maps, rows_per_expert, rt


def assemble_output(results, rows_per_expert, n_rows, index=None, b=None):
    y = np.zeros((n_rows, D_OUT), np.float32)
    for e, rows in enumerate(rows_per_expert):
        yc = results[e]["y"].reshape(-1, D_OUT)
        y[rows] = yc[: len(rows)]
    if b is not None and np.any(b):
        y += np.asarray(b, np.float32)[np.asarray(index)]
    return y


def kernel(x, index, W, b):
    x = np.asarray(x)
    index = np.asarray(index, np.int32)
    W = np.asarray(W)
    b = np.asarray(b)
    in_maps, rows_per_expert, rt = make_in_maps(x, index, W)
    nc = build_nc(rt)
    res = run_bass_kernel_spmd(nc, in_maps, core_ids=list(range(N_CORES)))
    return assemble_output(res.results, rows_per_expert, x.shape[0], index, b)


# revision 12
# speedup vs baseline: 1.5441x; 1.0163x over previous
"""MoE routing kernel for Trainium2 (8 NeuronCores, expert-parallel).

Problem: y[n] = x[n] @ W[index[n]].T + b[index[n]]
  x [16384, 1024] f32, index [16384] i32, W [8, 512, 1024] f32, b [8, 512] f32

Strategy (expert-parallel, dispatch on index during sharding):
  Core e owns expert e. The host groups rows by expert (the all-to-all
  dispatch), packs each core's rows into PE-friendly transposed tiles, and
  each core runs a dense [R,1024] @ [1024,512] matmul with its expert's
  weights. Results are scattered back to original row order on the host.

Device layout per core (one NEFF, SPMD on cores 0-7):
  xT  [RT, 128, 8, 128]  (row-tile, k%128, k-tile, r) — lhsT blocks; the
                         partition line (fixed k) is 4KB contiguous in DRAM
  wT  [8, 128, 512]      (k-tile, k, o)               — rhs blocks (moving)
  y   [RT, 128, 512]     (row-tile, r, o)
  For each row-tile: accumulate 8 matmuls over k-tiles into one PSUM bank,
  copy PSUM->SBUF on DVE, DMA out.
"""

from contextlib import ExitStack

import numpy as np

import concourse.bass as bass
import concourse.mybir as mybir
import concourse.tile as tile
from concourse import bacc
from concourse.bass_utils import run_bass_kernel_spmd

N_CORES = 8
D_IN = 1024
D_OUT = 512
KT = D_IN // 128  # 8 k-tiles

# matmul input dtypes (lhsT = x blocks, rhs = W blocks):
#   float32r streams 1 column/cycle (vs 4 cycles for float32) at N>=256;
#   bits are identical to fp32. bfloat16 additionally gets fast weight load.
X_DT = mybir.dt.float32r
W_DT = mybir.dt.float32r


def build_nc(rt: int, x_dt=None, w_dt=None):
    """Build + compile the per-core Bass program for `rt` row-tiles."""
    x_dt = x_dt or X_DT
    w_dt = w_dt or W_DT
    nc = bacc.Bacc(
        "TRN2", target_bir_lowering=False, debug=False, num_devices=N_CORES
    )
    f32 = mybir.dt.float32
    xT = nc.dram_tensor("xT", [rt, 128, KT * 128], x_dt, kind="ExternalInput").ap()
    wT = nc.dram_tensor("wT", [KT, 128, D_OUT], w_dt, kind="ExternalInput").ap()
    y = nc.dram_tensor("y", [rt, 128, D_OUT], f32, kind="ExternalOutput").ap()

    with tile.TileContext(nc) as tc, ExitStack() as ctx:
        w_pool = ctx.enter_context(tc.tile_pool(name="w", bufs=1))
        x_pool = ctx.enter_context(tc.tile_pool(name="x", bufs=6))
        o_pool = ctx.enter_context(tc.tile_pool(name="o", bufs=6))
        p_pool = ctx.enter_context(tc.tile_pool(name="p", bufs=6, space="PSUM"))

        # The ACT HWDGE ring (nc.scalar) finishes its entry protocol ~3us
        # before the SP ring (nc.sync), so the DMAs gating the first matmul
        # (x row-tile 0 + all of W) go on scalar; the remaining x row-tiles
        # stream on sync in parallel. Output stores also go on scalar, which
        # is idle after the W loads.
        w_tiles = []
        for kt in range(KT):
            w_tiles.append(
                w_pool.tile([128, D_OUT], w_dt, tag=f"w{kt}", name=f"w{kt}")
            )

        x_tiles = {}
        x_t0 = x_pool.tile([128, KT * 128], x_dt, tag="x", name="x0")
        nc.scalar.dma_start(x_t0[:], xT[0])
        x_tiles[0] = x_t0
        for kt in range(KT):
            nc.scalar.dma_start(w_tiles[kt][:], wT[kt])
        for r in range(1, rt):
            x_t = x_pool.tile([128, KT * 128], x_dt, tag="x", name=f"x{r}")
            nc.sync.dma_start(x_t[:], xT[r])
            x_tiles[r] = x_t

        for r in range(rt):
            x_t = x_tiles[r]
            psum = p_pool.tile([128, D_OUT], f32, tag="ps")
            for kt in range(KT):
                nc.tensor.matmul(
                    psum[:],
                    x_t[:, bass.ts(kt, 128)],
                    w_tiles[kt][:],
                    start=(kt == 0),
                    stop=(kt == KT - 1),
                )
            o_t = o_pool.tile([128, D_OUT], f32, tag="o")
            nc.vector.tensor_copy(o_t[:], psum[:])
            nc.scalar.dma_start(y[r], o_t[:])

    nc.compile()
    return nc


def make_in_maps(x, index, W, x_dt=None, w_dt=None):
    """Group rows by expert, pack per-core transposed tiles.

    Returns (in_maps, rows_per_expert, rt) where rows_per_expert[e] is the
    original row indices handled by core e.
    """
    import concourse.mybir as _mybir

    x_np = _mybir.dt.np(x_dt or X_DT)
    w_np = _mybir.dt.np(w_dt or W_DT)
    x = np.ascontiguousarray(x, dtype=np.float32)
    W = np.ascontiguousarray(W, dtype=np.float32)
    rows_per_expert = [np.nonzero(index == e)[0] for e in range(N_CORES)]
    max_rows = max(len(r) for r in rows_per_expert)
    rt = max((max_rows + 127) // 128, 1)
    r_pad = rt * 128

    in_maps = []
    for e in range(N_CORES):
        rows = rows_per_expert[e]
        xp = np.zeros((r_pad, D_IN), np.float32)
        xp[: len(rows)] = x[rows]
        # [R, D_IN] -> [RT, 128r, KT, 128k] -> [RT, 128k, KT, 128r]
        # so a partition line (fixed k) is KT*128*4B = 4KB contiguous.
        xT = np.ascontiguousarray(
            xp.reshape(rt, 128, KT, 128).transpose(0, 3, 2, 1).reshape(rt, 128, -1),
            dtype=x_np,
        )
        wT = np.ascontiguousarray(W[e].T.reshape(KT, 128, D_OUT), dtype=w_np)
        in_maps.append({"xT": xT, "wT": wT})
    return in_maps, rows_per_expert, rt


def assemble_output(results, rows_per_expert, n_rows, index=None, b=None):
    y = np.zeros((n_rows, D_OUT), np.float32)
    for e, rows in enumerate(rows_per_expert):
        yc = results[e]["y"].reshape(-1, D_OUT)
        y[rows] = yc[: len(rows)]
    if b is not None and np.any(b):
        y += np.asarray(b, np.float32)[np.asarray(index)]
    return y


def kernel(x, index, W, b):
    x = np.asarray(x)
    index = np.asarray(index, np.int32)
    W = np.asarray(W)
    b = np.asarray(b)
    in_maps, rows_per_expert, rt = make_in_maps(x, index, W)
    nc = build_nc(rt)
    res = run_bass_kernel_spmd(nc, in_maps, core_ids=list(range(N_CORES)))
    return assemble_output(res.results, rows_per_expert, x.shape[0], index, b)
